# revision 8
# baseline (speedup 1.0000x reference)
"""Trainium2 Bass kernel for BigramHashEmbedding.

reference:
    prev = shift_right(input_ids)                   # per batch row, first pos = 0
    idx  = (prev * 1000003 + input_ids) % 131072
    h    = table[idx]                               # [b, s, 2048] gather
    out  = einsum('bsh,dh->bsd', h, proj_w)         # [b, s, 6144]

Strategy (8 NeuronCores, data-parallel over the 16384 tokens):
  host: compute bigram indices (trivial int math), cast table/proj to bf16,
        pre-layout projT into contiguous [128, H/128, 512] blocks.
  core: 2048 tokens each.
        - indirect-DMA gather of 128 table rows at a time  -> h_nat [128 tok, 2048] bf16
        - xbar DMA transpose                                -> hT    [128 hid, 16, 128 tok]
        - PE matmul out[tok, d] = sum_k hT[k, tok] * projT[k, d]  (bf16 x bf16 -> fp32 PSUM)
        - ACT copies PSUM->SBUF, DMA stores to out [2048, 6144] fp32
  host: concat core outputs -> [4, 4096, 6144] fp32.

DMA ring assignment: gathers + proj loads on gpsimd (SWDGE); xbar transposes
alone on the sync HWDGE ring (no DMATranspose<->DMACopy mode flips); output
stores on the scalar HWDGE ring.
"""

import os
import sys
from contextlib import ExitStack

import numpy as np

for _p in ("/opt/trn_rl_repo", "/root/.axon_site/_ro/trn_rl_repo"):
    if os.path.isdir(_p) and _p not in sys.path:
        sys.path.insert(0, _p)

import ml_dtypes

import concourse.bass as bass
import concourse.tile as tile
from concourse import bacc, mybir
from concourse.bass_utils import run_bass_kernel_spmd

BIGRAM_VOCAB = 131072
BIGRAM_HIDDEN = 2048
MODEL_DIM = 6144
HASH_MULT = 1000003
N_CORES = 8
P = 128

# last BassKernelResults (for profiling from test harnesses)
LAST_RESULT = None


def build_kernel(
    V=BIGRAM_VOCAB,
    H=BIGRAM_HIDDEN,
    D=MODEL_DIM,
    T=2048,
    d_pass=1536,
    reps=1,
    order="j_outer",
    prologue="v3",
    out_dt="f32",
):
    """Build the per-core Bass program (SPMD: same program, per-core inputs).

    V: vocab rows in the (bf16) table
    H: hidden size of a table row        (H % 128 == 0)
    D: output model dim                  (D % d_pass == 0)
    T: tokens handled by this core       (T % 128 == 0)
    d_pass: D-columns resident per pass  (d_pass % 512 == 0)
    order: "j_outer" (one psum chain at a time) or
           "c_outer" (n_j chains interleaved; each lhsT reused n_j times)
    """
    n_tok = T // P
    n_chunks = H // P
    n_pass = D // d_pass
    n_j = d_pass // 512
    n_blk = n_pass * n_j  # total 512-wide D blocks
    bf16 = mybir.dt.bfloat16
    f32 = mybir.dt.float32
    out_mydt = bf16 if out_dt == "bf16" else f32

    nc = bacc.Bacc("TRN2", target_bir_lowering=False, debug=False)
    idx_d = nc.dram_tensor("idx", [P, n_tok], mybir.dt.int32, kind="ExternalInput")
    table_d = nc.dram_tensor("table", [V, H], bf16, kind="ExternalInput")
    projT_d = nc.dram_tensor(
        "projT", [n_blk, P, n_chunks, 512], bf16, kind="ExternalInput"
    )
    out_d = nc.dram_tensor("out", [T, D], out_mydt, kind="ExternalOutput")

    with tile.TileContext(nc) as tc, ExitStack() as ctx:
        idx_pool = ctx.enter_context(tc.tile_pool(name="idx", bufs=1))
        hnat_pool = ctx.enter_context(tc.tile_pool(name="hnat", bufs=2))
        ht_pool = ctx.enter_context(tc.tile_pool(name="ht", bufs=1))
        proj_pool = ctx.enter_context(tc.tile_pool(name="proj", bufs=2))
        osb_pool = ctx.enter_context(tc.tile_pool(name="osb", bufs=6))
        psum_bufs = 8 if order == "j_outer" else 2
        psum_pool = ctx.enter_context(
            tc.tile_pool(name="psum", bufs=psum_bufs, space="PSUM")
        )

        idx_sb = idx_pool.tile([P, n_tok], mybir.dt.int32)
        nc.sync.dma_start(idx_sb[:], idx_d[:])

        for _rep in range(reps):
            _kernel_body(
                nc, tc, idx_sb, table_d, projT_d, out_d,
                n_tok, n_chunks, n_pass, n_j, d_pass, H,
                hnat_pool, ht_pool, proj_pool, osb_pool, psum_pool, _rep,
                order, prologue, out_mydt,
            )

    nc.compile()
    return nc


def _kernel_body(
    nc, tc, idx_sb, table_d, projT_d, out_d,
    n_tok, n_chunks, n_pass, n_j, d_pass, H,
    hnat_pool, ht_pool, proj_pool, osb_pool, psum_pool, rep,
    order="j_outer",
    prologue="v3",
    out_mydt=None,
):
    bf16 = mybir.dt.bfloat16
    f32 = mybir.dt.float32
    if out_mydt is None:
        out_mydt = f32

    # First-pass proj blocks load on the scalar HWDGE ring (idle until the
    # first PSUM drain ~25us in) and are emitted BEFORE the gathers, so the
    # first matmul only waits ~one gather + one transpose instead of queuing
    # behind all 16 gather emissions on the SWDGE ring.
    projs_q0 = []
    if prologue == "v3":
        for j in range(n_j):
            pj = proj_pool.tile(
                [P, n_chunks, 512], bf16, tag=f"proj{j}", name=f"proj_{rep}_0_{j}"
            )
            nc.scalar.dma_start(pj[:], projT_d[j])
            projs_q0.append(pj)

    # gather + transpose each 128-token tile
    hts = []
    for i in range(n_tok):
        h_nat = hnat_pool.tile([P, H], bf16, tag="hnat", name=f"hnat_{rep}_{i}")
        nc.gpsimd.indirect_dma_start(
            out=h_nat[:],
            out_offset=None,
            in_=table_d[:, :],
            in_offset=bass.IndirectOffsetOnAxis(ap=idx_sb[:, i : i + 1], axis=0),
        )
        ht = ht_pool.tile([P, n_chunks, P], bf16, tag=f"ht{i}", name=f"ht_{rep}_{i}")
        nc.sync.dma_start_transpose(ht[:], h_nat[:])
        hts.append(ht)

    for q in range(n_pass):
        if q == 0 and prologue == "v3":
            projs = projs_q0
        else:
            projs = []
            for j in range(n_j):
                pj = proj_pool.tile(
                    [P, n_chunks, 512], bf16, tag=f"proj{j}", name=f"proj_{rep}_{q}_{j}"
                )
                nc.gpsimd.dma_start(pj[:], projT_d[q * n_j + j])
                projs.append(pj)
        for i in range(n_tok):
            if order == "j_outer":
                for j in range(n_j):
                    ps = psum_pool.tile(
                        [P, 512], f32, tag="ps", name=f"ps_{rep}_{q}_{i}_{j}"
                    )
                    for c in range(n_chunks):
                        nc.tensor.matmul(
                            ps[:],
                            hts[i][:, c, :],
                            projs[j][:, c, :],
                            start=(c == 0),
                            stop=(c == n_chunks - 1),
                        )
                    osb = osb_pool.tile(
                        [P, 512], out_mydt, tag="osb", name=f"osb_{rep}_{q}_{i}_{j}"
                    )
                    nc.scalar.copy(osb[:], ps[:])
                    col0 = q * d_pass + j * 512
                    nc.scalar.dma_start(
                        out_d[i * P : (i + 1) * P, col0 : col0 + 512], osb[:]
                    )
            else:  # c_outer: each lhsT load feeds n_j consecutive matmuls
                pss = [
                    psum_pool.tile(
                        [P, 512], f32, tag=f"ps{j}", name=f"ps_{rep}_{q}_{i}_{j}"
                    )
                    for j in range(n_j)
                ]
                for c in range(n_chunks):
                    for j in range(n_j):
                        nc.tensor.matmul(
                            pss[j][:],
                            hts[i][:, c, :],
                            projs[j][:, c, :],
                            start=(c == 0),
                            stop=(c == n_chunks - 1),
                        )
                for j in range(n_j):
                    osb = osb_pool.tile(
                        [P, 512], out_mydt, tag="osb", name=f"osb_{rep}_{q}_{i}_{j}"
                    )
                    nc.scalar.copy(osb[:], pss[j][:])
                    col0 = q * d_pass + j * 512
                    nc.scalar.dma_start(
                        out_d[i * P : (i + 1) * P, col0 : col0 + 512], osb[:]
                    )


_NC_CACHE = {}


def _get_nc(key, **kwargs):
    if key not in _NC_CACHE:
        _NC_CACHE[key] = build_kernel(**kwargs)
    return _NC_CACHE[key]


def _bigram_indices(input_ids):
    ids = np.asarray(input_ids).astype(np.int64)
    prev = np.concatenate([np.zeros_like(ids[:, :1]), ids[:, :-1]], axis=1)
    return ((prev * HASH_MULT + ids) % BIGRAM_VOCAB).astype(np.int32)


def _prep_proj(proj_w, d_pass=1536):
    """[D, H] f32 -> [n_blk, 128, H/128, 512] bf16 contiguous blocks."""
    Hh = proj_w.shape[1]
    D = proj_w.shape[0]
    n_chunks = Hh // P
    projT = np.ascontiguousarray(np.asarray(proj_w).T)  # [H, D]
    # [H, D] -> [128, n_chunks, D]: partition p holds rows {c*128 + p}
    a = projT.reshape(n_chunks, P, D).transpose(1, 0, 2)
    # -> [n_blk, 128, n_chunks, 512]
    a = a.reshape(P, n_chunks, D // 512, 512).transpose(2, 0, 1, 3)
    return np.ascontiguousarray(a).astype(ml_dtypes.bfloat16)


# Best-known device config (bench-validated). kernel() always uses this.
BEST_CONFIG = dict(d_pass=1536, order="j_outer", prologue="v3", out_dt="bf16")


def kernel(input_ids, table, proj_w):
    global LAST_RESULT
    b, s = input_ids.shape
    n_tokens = b * s
    T = n_tokens // N_CORES
    assert T % P == 0

    nc = _get_nc(("main", T, tuple(sorted(BEST_CONFIG.items()))), T=T, **BEST_CONFIG)

    flat_idx = _bigram_indices(input_ids).reshape(-1)
    table_bf = np.asarray(table, dtype=ml_dtypes.bfloat16)
    projT_prep = _prep_proj(proj_w)

    in_maps = []
    for ci in range(N_CORES):
        sl = flat_idx[ci * T : (ci + 1) * T]
        idx_np = np.ascontiguousarray(sl.reshape(T // P, P).T).astype(np.int32)
        in_maps.append({"idx": idx_np, "table": table_bf, "projT": projT_prep})

    want_trace = bool(int(os.environ.get("KERNEL_TRACE", "0")))
    if not want_trace:
        # This axon build lacks the NTFF profile hook (antenv.axon_hooks);
        # run_bass_kernel_spmd's trace path would crash on import if the
        # environment sets BASS_TRACE. Force the plain execute path.
        os.environ["BASS_NEVER_TRACE"] = "1"
    LAST_RESULT = run_bass_kernel_spmd(
        nc,
        in_maps,
        core_ids=list(range(N_CORES)),
        trace=want_trace,
    )
    out = np.concatenate(
        [np.asarray(r["out"]).astype(np.float32) for r in LAST_RESULT.results], axis=0
    )
    return out.reshape(b, s, MODEL_DIM)



# revision 16
# speedup vs baseline: 1.2490x; 1.2490x over previous
"""Trainium2 Bass kernel for BigramHashEmbedding.

reference:
    prev = shift_right(input_ids)                   # per batch row, first pos = 0
    idx  = (prev * 1000003 + input_ids) % 131072
    h    = table[idx]                               # [b, s, 2048] gather
    out  = einsum('bsh,dh->bsd', h, proj_w)         # [b, s, 6144]

Strategy (8 NeuronCores, data-parallel over the 16384 tokens):
  host: compute bigram indices (trivial int math), cast table/proj to bf16,
        pre-layout projT into contiguous [128, H/128, 512] blocks.
  core: 2048 tokens each.
        - indirect-DMA gather of 128 table rows at a time  -> h_nat [128 tok, 2048] bf16
        - xbar DMA transpose                                -> hT    [128 hid, 16, 128 tok]
        - PE matmul out[tok, d] = sum_k hT[k, tok] * projT[k, d]  (bf16 x bf16 -> fp32 PSUM)
        - ACT copies PSUM->SBUF casting to bf16, DMA stores out [2048, 6144] bf16
  host: concat core outputs, upcast bf16 -> [4, 4096, 6144] fp32.

DMA ring assignment: gathers + proj loads on gpsimd (SWDGE); xbar transposes
alone on the sync HWDGE ring (no DMATranspose<->DMACopy mode flips); output
stores on the scalar HWDGE ring.

Measured constraints (see memory notes): the kernel is PE-bound — 3072
N=512 bf16 matmuls/core at 1 column/cycle @2.4GHz = 655us hard floor; the
kernel steady-state measures ~570-650us/rep (delta-reps, interleaved).
fp8 (DoubleRow, the only 2x PE lever) fails the 2e-2 rel-err gate: e4m3
quantization alone costs 2.7-3.7% rel_l2. bf16 output store (vs f32)
halves store traffic and measured fastest; alternative loop orders
(c_outer LDW amortization, j_sweep single-block startup) measured equal
or slower on HW.
"""

import os
import sys
from contextlib import ExitStack

import numpy as np

for _p in ("/opt/trn_rl_repo", "/root/.axon_site/_ro/trn_rl_repo"):
    if os.path.isdir(_p) and _p not in sys.path:
        sys.path.insert(0, _p)

import ml_dtypes

import concourse.bass as bass
import concourse.tile as tile
from concourse import bacc, mybir
from concourse.bass_utils import run_bass_kernel_spmd

BIGRAM_VOCAB = 131072
BIGRAM_HIDDEN = 2048
MODEL_DIM = 6144
HASH_MULT = 1000003
N_CORES = 8
P = 128

# last BassKernelResults (for profiling from test harnesses)
LAST_RESULT = None


def build_kernel(
    V=BIGRAM_VOCAB,
    H=BIGRAM_HIDDEN,
    D=MODEL_DIM,
    T=2048,
    d_pass=1536,
    reps=1,
    order="j_outer",
    prologue="v3",
    out_dt="f32",
    scratch=16384,
):
    """Build the per-core Bass program (SPMD: same program, per-core inputs).

    V: vocab rows in the (bf16) table
    H: hidden size of a table row        (H % 128 == 0)
    D: output model dim                  (D % d_pass == 0)
    T: tokens handled by this core       (T % 128 == 0)
    d_pass: D-columns resident per pass  (d_pass % 512 == 0)
    order: "j_outer" (one psum chain at a time) or
           "c_outer" (n_j chains interleaved; each lhsT reused n_j times)
    """
    n_tok = T // P
    n_chunks = H // P
    n_pass = D // d_pass
    n_j = d_pass // 512
    n_blk = n_pass * n_j  # total 512-wide D blocks
    bf16 = mybir.dt.bfloat16
    f32 = mybir.dt.float32
    out_mydt = bf16 if out_dt == "bf16" else f32

    nc = bacc.Bacc(
        "TRN2",
        target_bir_lowering=False,
        debug=False,
        dynamic_dma_scratch_size=scratch,
    )
    idx_d = nc.dram_tensor("idx", [P, n_tok], mybir.dt.int32, kind="ExternalInput")
    table_d = nc.dram_tensor("table", [V, H], bf16, kind="ExternalInput")
    projT_d = nc.dram_tensor(
        "projT", [n_blk, P, n_chunks, 512], bf16, kind="ExternalInput"
    )
    out_d = nc.dram_tensor("out", [T, D], out_mydt, kind="ExternalOutput")

    with tile.TileContext(nc) as tc, ExitStack() as ctx:
        idx_pool = ctx.enter_context(tc.tile_pool(name="idx", bufs=1))
        hnat_pool = ctx.enter_context(tc.tile_pool(name="hnat", bufs=2))
        ht_pool = ctx.enter_context(tc.tile_pool(name="ht", bufs=1))
        proj_pool = ctx.enter_context(tc.tile_pool(name="proj", bufs=2))
        osb_pool = ctx.enter_context(tc.tile_pool(name="osb", bufs=6))
        psum_bufs = 8 if order == "j_outer" else 2
        psum_pool = ctx.enter_context(
            tc.tile_pool(name="psum", bufs=psum_bufs, space="PSUM")
        )

        idx_sb = idx_pool.tile([P, n_tok], mybir.dt.int32)
        nc.sync.dma_start(idx_sb[:], idx_d[:])

        for _rep in range(reps):
            _kernel_body(
                nc, tc, idx_sb, table_d, projT_d, out_d,
                n_tok, n_chunks, n_pass, n_j, d_pass, H,
                hnat_pool, ht_pool, proj_pool, osb_pool, psum_pool, _rep,
                order, prologue, out_mydt,
            )

    nc.compile()
    return nc


def _kernel_body(
    nc, tc, idx_sb, table_d, projT_d, out_d,
    n_tok, n_chunks, n_pass, n_j, d_pass, H,
    hnat_pool, ht_pool, proj_pool, osb_pool, psum_pool, rep,
    order="j_outer",
    prologue="v3",
    out_mydt=None,
):
    bf16 = mybir.dt.bfloat16
    f32 = mybir.dt.float32
    if out_mydt is None:
        out_mydt = f32

    # First-pass proj blocks load on the scalar HWDGE ring (idle until the
    # first PSUM drain ~25us in) and are emitted BEFORE the gathers, so the
    # first matmul only waits ~one gather + one transpose instead of queuing
    # behind all 16 gather emissions on the SWDGE ring.
    projs_q0 = []
    if prologue in ("v3", "v4"):
        for j in range(n_j):
            pj = proj_pool.tile(
                [P, n_chunks, 512], bf16, tag=f"proj{j}", name=f"proj_{rep}_0_{j}"
            )
            nc.scalar.dma_start(pj[:], projT_d[j])
            projs_q0.append(pj)
    elif prologue == "v5":
        # Only block 0 up front (the sole prerequisite of the first matmul
        # chain under j_sweep) — loading all n_j blocks first costs 3x the
        # HBM bytes on the critical path to the first matmul.
        pj = proj_pool.tile(
            [P, n_chunks, 512], bf16, tag="proj0", name=f"proj_{rep}_k0"
        )
        nc.scalar.dma_start(pj[:], projT_d[0])
        projs_q0.append(pj)

    # gather + transpose each 128-token tile
    hts = []
    for i in range(n_tok):
        h_nat = hnat_pool.tile([P, H], bf16, tag="hnat", name=f"hnat_{rep}_{i}")
        nc.gpsimd.indirect_dma_start(
            out=h_nat[:],
            out_offset=None,
            in_=table_d[:, :],
            in_offset=bass.IndirectOffsetOnAxis(ap=idx_sb[:, i : i + 1], axis=0),
        )
        ht = ht_pool.tile([P, n_chunks, P], bf16, tag=f"ht{i}", name=f"ht_{rep}_{i}")
        nc.sync.dma_start_transpose(ht[:], h_nat[:])
        hts.append(ht)
        if prologue == "v5":
            # blocks 1..2 ride the sync ring between transposes: needed at
            # ~60us/~115us, landing ~15us/~25us; later transposes slip a few
            # us but stay well ahead of their ~3.4us-per-tile consumption.
            k_next = len(projs_q0)
            if (i == 4 and k_next == 1) or (i == 8 and k_next == 2):
                pj = proj_pool.tile(
                    [P, n_chunks, 512],
                    bf16,
                    tag=f"proj{k_next % n_j}",
                    name=f"proj_{rep}_k{k_next}",
                )
                nc.sync.dma_start(pj[:], projT_d[k_next])
                projs_q0.append(pj)

    if order == "j_sweep":
        # Flat 512-wide D-block sweep: block k serves all n_tok token tiles
        # before block k+1 starts. Only ONE proj block is needed for the
        # first ~n_tok*3.4us of matmuls, so the PE reaches full rate ~6us in
        # (vs ~17us when the first token tile needs all n_j blocks).
        # Block loads: 0..n_j-1 via the scalar ring (prologue, emitted
        # above); the rest on the sync ring AFTER the transposes ("v4") or
        # on gpsimd behind the gathers (legacy). Each block is consumed
        # ~n_tok*3.4us after the previous — far behind either load rate,
        # and none of them queue behind the 2048 gather descriptors on the
        # SWDGE ring.
        n_blk = n_pass * n_j
        blocks = list(projs_q0)
        ring = nc.sync if prologue in ("v4", "v5") else nc.gpsimd
        for k in range(len(blocks), n_blk):
            pj = proj_pool.tile(
                [P, n_chunks, 512], bf16, tag=f"proj{k % n_j}", name=f"proj_{rep}_k{k}"
            )
            ring.dma_start(pj[:], projT_d[k])
            blocks.append(pj)
        for k in range(n_blk):
            pj = blocks[k]
            col0 = k * 512
            for i in range(n_tok):
                ps = psum_pool.tile(
                    [P, 512], f32, tag="ps", name=f"ps_{rep}_{k}_{i}"
                )
                for c in range(n_chunks):
                    nc.tensor.matmul(
                        ps[:],
                        hts[i][:, c, :],
                        pj[:, c, :],
                        start=(c == 0),
                        stop=(c == n_chunks - 1),
                    )
                osb = osb_pool.tile(
                    [P, 512], out_mydt, tag="osb", name=f"osb_{rep}_{k}_{i}"
                )
                nc.scalar.copy(osb[:], ps[:])
                nc.scalar.dma_start(
                    out_d[i * P : (i + 1) * P, col0 : col0 + 512], osb[:]
                )
        return

    for q in range(n_pass):
        if q == 0 and prologue == "v3":
            projs = projs_q0
        else:
            projs = []
            for j in range(n_j):
                pj = proj_pool.tile(
                    [P, n_chunks, 512], bf16, tag=f"proj{j}", name=f"proj_{rep}_{q}_{j}"
                )
                nc.gpsimd.dma_start(pj[:], projT_d[q * n_j + j])
                projs.append(pj)
        for i in range(n_tok):
            if order == "j_outer":
                for j in range(n_j):
                    ps = psum_pool.tile(
                        [P, 512], f32, tag="ps", name=f"ps_{rep}_{q}_{i}_{j}"
                    )
                    for c in range(n_chunks):
                        nc.tensor.matmul(
                            ps[:],
                            hts[i][:, c, :],
                            projs[j][:, c, :],
                            start=(c == 0),
                            stop=(c == n_chunks - 1),
                        )
                    osb = osb_pool.tile(
                        [P, 512], out_mydt, tag="osb", name=f"osb_{rep}_{q}_{i}_{j}"
                    )
                    nc.scalar.copy(osb[:], ps[:])
                    col0 = q * d_pass + j * 512
                    nc.scalar.dma_start(
                        out_d[i * P : (i + 1) * P, col0 : col0 + 512], osb[:]
                    )
            else:  # c_outer: each lhsT load feeds n_j consecutive matmuls
                pss = [
                    psum_pool.tile(
                        [P, 512], f32, tag=f"ps{j}", name=f"ps_{rep}_{q}_{i}_{j}"
                    )
                    for j in range(n_j)
                ]
                for c in range(n_chunks):
                    for j in range(n_j):
                        nc.tensor.matmul(
                            pss[j][:],
                            hts[i][:, c, :],
                            projs[j][:, c, :],
                            start=(c == 0),
                            stop=(c == n_chunks - 1),
                        )
                for j in range(n_j):
                    osb = osb_pool.tile(
                        [P, 512], out_mydt, tag="osb", name=f"osb_{rep}_{q}_{i}_{j}"
                    )
                    nc.scalar.copy(osb[:], pss[j][:])
                    col0 = q * d_pass + j * 512
                    nc.scalar.dma_start(
                        out_d[i * P : (i + 1) * P, col0 : col0 + 512], osb[:]
                    )


_NC_CACHE = {}


def _get_nc(key, **kwargs):
    if key not in _NC_CACHE:
        _NC_CACHE[key] = build_kernel(**kwargs)
    return _NC_CACHE[key]


def _bigram_indices(input_ids):
    ids = np.asarray(input_ids).astype(np.int64)
    prev = np.concatenate([np.zeros_like(ids[:, :1]), ids[:, :-1]], axis=1)
    return ((prev * HASH_MULT + ids) % BIGRAM_VOCAB).astype(np.int32)


def _prep_proj(proj_w, d_pass=1536):
    """[D, H] f32 -> [n_blk, 128, H/128, 512] bf16 contiguous blocks."""
    Hh = proj_w.shape[1]
    D = proj_w.shape[0]
    n_chunks = Hh // P
    projT = np.ascontiguousarray(np.asarray(proj_w).T)  # [H, D]
    # [H, D] -> [128, n_chunks, D]: partition p holds rows {c*128 + p}
    a = projT.reshape(n_chunks, P, D).transpose(1, 0, 2)
    # -> [n_blk, 128, n_chunks, 512]
    a = a.reshape(P, n_chunks, D // 512, 512).transpose(2, 0, 1, 3)
    return np.ascontiguousarray(a).astype(ml_dtypes.bfloat16)


# Best-known device config (bench-validated). kernel() always uses this.
BEST_CONFIG = dict(d_pass=1536, order="j_outer", prologue="v3", out_dt="bf16")


def kernel(input_ids, table, proj_w):
    global LAST_RESULT
    b, s = input_ids.shape
    n_tokens = b * s
    T = n_tokens // N_CORES
    assert T % P == 0

    nc = _get_nc(("main", T, tuple(sorted(BEST_CONFIG.items()))), T=T, **BEST_CONFIG)

    flat_idx = _bigram_indices(input_ids).reshape(-1)
    table_bf = np.asarray(table, dtype=ml_dtypes.bfloat16)
    projT_prep = _prep_proj(proj_w)

    in_maps = []
    for ci in range(N_CORES):
        sl = flat_idx[ci * T : (ci + 1) * T]
        idx_np = np.ascontiguousarray(sl.reshape(T // P, P).T).astype(np.int32)
        in_maps.append({"idx": idx_np, "table": table_bf, "projT": projT_prep})

    want_trace = bool(int(os.environ.get("KERNEL_TRACE", "0")))
    if not want_trace:
        # This axon build lacks the NTFF profile hook (antenv.axon_hooks);
        # run_bass_kernel_spmd's trace path would crash on import if the
        # environment sets BASS_TRACE. Force the plain execute path.
        os.environ["BASS_NEVER_TRACE"] = "1"
    LAST_RESULT = run_bass_kernel_spmd(
        nc,
        in_maps,
        core_ids=list(range(N_CORES)),
        trace=want_trace,
    )
    out = np.concatenate(
        [np.asarray(r["out"]).astype(np.float32) for r in LAST_RESULT.results], axis=0
    )
    return out.reshape(b, s, MODEL_DIM)



# revision 21
# speedup vs baseline: 1.3165x; 1.0540x over previous
"""Trainium2 Bass kernel for BigramHashEmbedding.

reference:
    prev = shift_right(input_ids)                   # per batch row, first pos = 0
    idx  = (prev * 1000003 + input_ids) % 131072
    h    = table[idx]                               # [b, s, 2048] gather
    out  = einsum('bsh,dh->bsd', h, proj_w)         # [b, s, 6144]

Strategy (8 NeuronCores, data-parallel over the 16384 tokens):
  host: compute bigram indices (trivial int math), cast table/proj to bf16,
        pre-layout projT into contiguous [128, H/128, 512] blocks.
  core: 2048 tokens each.
        - indirect-DMA gather of 128 table rows at a time  -> h_nat [128 tok, 2048] bf16
        - xbar DMA transpose                                -> hT    [128 hid, 16, 128 tok]
        - PE matmul out[tok, d] = sum_k hT[k, tok] * projT[k, d]  (bf16 x bf16 -> fp32 PSUM)
        - ACT copies PSUM->SBUF casting to bf16, DMA stores out [2048, 6144] bf16
  host: concat core outputs, upcast bf16 -> [4, 4096, 6144] fp32.

DMA ring assignment: gathers + proj loads on gpsimd (SWDGE); xbar transposes
alone on the sync HWDGE ring (no DMATranspose<->DMACopy mode flips); output
stores on the scalar HWDGE ring.

Measured constraints (see memory notes): the kernel is PE-bound — 3072
N=512 bf16 matmuls/core at 1 column/cycle @2.4GHz = 655us hard floor; the
kernel steady-state measures ~570-650us/rep (delta-reps, interleaved).
fp8 (DoubleRow, the only 2x PE lever) fails the 2e-2 rel-err gate: e4m3
quantization alone costs 2.7-3.7% rel_l2. bf16 output store (vs f32)
halves store traffic and measured fastest; alternative loop orders
(c_outer LDW amortization, j_sweep single-block startup) measured equal
or slower on HW.
"""

import os
import sys
from contextlib import ExitStack

import numpy as np

for _p in ("/opt/trn_rl_repo", "/root/.axon_site/_ro/trn_rl_repo"):
    if os.path.isdir(_p) and _p not in sys.path:
        sys.path.insert(0, _p)

import ml_dtypes

import concourse.bass as bass
import concourse.tile as tile
from concourse import bacc, mybir
from concourse.bass_utils import run_bass_kernel_spmd

BIGRAM_VOCAB = 131072
BIGRAM_HIDDEN = 2048
MODEL_DIM = 6144
HASH_MULT = 1000003
N_CORES = 8
P = 128

# last BassKernelResults (for profiling from test harnesses)
LAST_RESULT = None


def build_kernel(
    V=BIGRAM_VOCAB,
    H=BIGRAM_HIDDEN,
    D=MODEL_DIM,
    T=2048,
    d_pass=1536,
    reps=1,
    order="j_outer",
    prologue="v3",
    out_dt="f32",
    scratch=16384,
    drain="act",
    store_split=0,
    proj_ring="gpsimd",
):
    """Build the per-core Bass program (SPMD: same program, per-core inputs).

    V: vocab rows in the (bf16) table
    H: hidden size of a table row        (H % 128 == 0)
    D: output model dim                  (D % d_pass == 0)
    T: tokens handled by this core       (T % 128 == 0)
    d_pass: D-columns resident per pass  (d_pass % 512 == 0)
    order: "j_outer" (one psum chain at a time) or
           "c_outer" (n_j chains interleaved; each lhsT reused n_j times)
    """
    n_tok = T // P
    n_chunks = H // P
    n_pass = D // d_pass
    n_j = d_pass // 512
    n_blk = n_pass * n_j  # total 512-wide D blocks
    bf16 = mybir.dt.bfloat16
    f32 = mybir.dt.float32
    out_mydt = bf16 if out_dt == "bf16" else f32

    nc = bacc.Bacc(
        "TRN2",
        target_bir_lowering=False,
        debug=False,
        dynamic_dma_scratch_size=scratch,
    )
    idx_d = nc.dram_tensor("idx", [P, n_tok], mybir.dt.int32, kind="ExternalInput")
    table_d = nc.dram_tensor("table", [V, H], bf16, kind="ExternalInput")
    projT_d = nc.dram_tensor(
        "projT", [n_blk, P, n_chunks, 512], bf16, kind="ExternalInput"
    )
    out_d = nc.dram_tensor("out", [T, D], out_mydt, kind="ExternalOutput")

    with tile.TileContext(nc) as tc, ExitStack() as ctx:
        idx_pool = ctx.enter_context(tc.tile_pool(name="idx", bufs=1))
        hnat_pool = ctx.enter_context(tc.tile_pool(name="hnat", bufs=2))
        ht_pool = ctx.enter_context(tc.tile_pool(name="ht", bufs=1))
        proj_pool = ctx.enter_context(tc.tile_pool(name="proj", bufs=2))
        osb_pool = ctx.enter_context(tc.tile_pool(name="osb", bufs=6))
        psum_bufs = 8 if order == "j_outer" else 2
        psum_pool = ctx.enter_context(
            tc.tile_pool(name="psum", bufs=psum_bufs, space="PSUM")
        )

        idx_sb = idx_pool.tile([P, n_tok], mybir.dt.int32)
        nc.sync.dma_start(idx_sb[:], idx_d[:])

        for _rep in range(reps):
            _kernel_body(
                nc, tc, idx_sb, table_d, projT_d, out_d,
                n_tok, n_chunks, n_pass, n_j, d_pass, H,
                hnat_pool, ht_pool, proj_pool, osb_pool, psum_pool, _rep,
                order, prologue, out_mydt, drain, store_split, proj_ring,
            )

    nc.compile()
    return nc


def _kernel_body(
    nc, tc, idx_sb, table_d, projT_d, out_d,
    n_tok, n_chunks, n_pass, n_j, d_pass, H,
    hnat_pool, ht_pool, proj_pool, osb_pool, psum_pool, rep,
    order="j_outer",
    prologue="v3",
    out_mydt=None,
    drain="act",
    store_split=0,
    proj_ring="gpsimd",
):
    bf16 = mybir.dt.bfloat16
    f32 = mybir.dt.float32
    if out_mydt is None:
        out_mydt = f32

    def drain_copy(osb, ps):
        # PSUM -> SBUF drain (casts to osb dtype). "dve" uses the otherwise
        # idle Vector engine so the scalar engine only issues stores.
        if drain == "dve":
            nc.vector.tensor_copy(osb, ps)
        else:
            nc.scalar.copy(osb, ps)

    _store_n = [0]

    def store(dst_ap, osb):
        # optionally alternate output stores across both HWDGE rings
        # (sync ring is idle once the 16 transposes finish)
        if store_split and _store_n[0] % 2 == 1:
            nc.sync.dma_start(dst_ap, osb)
        else:
            nc.scalar.dma_start(dst_ap, osb)
        _store_n[0] += 1

    # First-pass proj blocks load on the scalar HWDGE ring (idle until the
    # first PSUM drain ~25us in) and are emitted BEFORE the gathers, so the
    # first matmul only waits ~one gather + one transpose instead of queuing
    # behind all 16 gather emissions on the SWDGE ring.
    projs_q0 = []
    if prologue in ("v3", "v4"):
        for j in range(n_j):
            pj = proj_pool.tile(
                [P, n_chunks, 512], bf16, tag=f"proj{j}", name=f"proj_{rep}_0_{j}"
            )
            nc.scalar.dma_start(pj[:], projT_d[j])
            projs_q0.append(pj)
    elif prologue == "v5":
        # Only block 0 up front (the sole prerequisite of the first matmul
        # chain under j_sweep) — loading all n_j blocks first costs 3x the
        # HBM bytes on the critical path to the first matmul.
        pj = proj_pool.tile(
            [P, n_chunks, 512], bf16, tag="proj0", name=f"proj_{rep}_k0"
        )
        nc.scalar.dma_start(pj[:], projT_d[0])
        projs_q0.append(pj)

    # gather + transpose each 128-token tile
    hts = []
    for i in range(n_tok):
        h_nat = hnat_pool.tile([P, H], bf16, tag="hnat", name=f"hnat_{rep}_{i}")
        nc.gpsimd.indirect_dma_start(
            out=h_nat[:],
            out_offset=None,
            in_=table_d[:, :],
            in_offset=bass.IndirectOffsetOnAxis(ap=idx_sb[:, i : i + 1], axis=0),
        )
        ht = ht_pool.tile([P, n_chunks, P], bf16, tag=f"ht{i}", name=f"ht_{rep}_{i}")
        nc.sync.dma_start_transpose(ht[:], h_nat[:])
        hts.append(ht)
        if prologue == "v5":
            # blocks 1..2 ride the sync ring between transposes: needed at
            # ~60us/~115us, landing ~15us/~25us; later transposes slip a few
            # us but stay well ahead of their ~3.4us-per-tile consumption.
            k_next = len(projs_q0)
            if (i == 4 and k_next == 1) or (i == 8 and k_next == 2):
                pj = proj_pool.tile(
                    [P, n_chunks, 512],
                    bf16,
                    tag=f"proj{k_next % n_j}",
                    name=f"proj_{rep}_k{k_next}",
                )
                nc.sync.dma_start(pj[:], projT_d[k_next])
                projs_q0.append(pj)

    if order == "j_sweep":
        # Flat 512-wide D-block sweep: block k serves all n_tok token tiles
        # before block k+1 starts. Only ONE proj block is needed for the
        # first ~n_tok*3.4us of matmuls, so the PE reaches full rate ~6us in
        # (vs ~17us when the first token tile needs all n_j blocks).
        # Block loads: 0..n_j-1 via the scalar ring (prologue, emitted
        # above); the rest on the sync ring AFTER the transposes ("v4") or
        # on gpsimd behind the gathers (legacy). Each block is consumed
        # ~n_tok*3.4us after the previous — far behind either load rate,
        # and none of them queue behind the 2048 gather descriptors on the
        # SWDGE ring.
        n_blk = n_pass * n_j
        blocks = list(projs_q0)
        ring = nc.sync if prologue in ("v4", "v5") else nc.gpsimd
        for k in range(len(blocks), n_blk):
            pj = proj_pool.tile(
                [P, n_chunks, 512], bf16, tag=f"proj{k % n_j}", name=f"proj_{rep}_k{k}"
            )
            ring.dma_start(pj[:], projT_d[k])
            blocks.append(pj)
        for k in range(n_blk):
            pj = blocks[k]
            col0 = k * 512
            for i in range(n_tok):
                ps = psum_pool.tile(
                    [P, 512], f32, tag="ps", name=f"ps_{rep}_{k}_{i}"
                )
                for c in range(n_chunks):
                    nc.tensor.matmul(
                        ps[:],
                        hts[i][:, c, :],
                        pj[:, c, :],
                        start=(c == 0),
                        stop=(c == n_chunks - 1),
                    )
                osb = osb_pool.tile(
                    [P, 512], out_mydt, tag="osb", name=f"osb_{rep}_{k}_{i}"
                )
                drain_copy(osb[:], ps[:])
                store(out_d[i * P : (i + 1) * P, col0 : col0 + 512], osb[:])
        return

    for q in range(n_pass):
        if q == 0 and prologue == "v3":
            projs = projs_q0
        else:
            projs = []
            for j in range(n_j):
                pj = proj_pool.tile(
                    [P, n_chunks, 512], bf16, tag=f"proj{j}", name=f"proj_{rep}_{q}_{j}"
                )
                (nc.sync if proj_ring == "sync" else nc.gpsimd).dma_start(
                    pj[:], projT_d[q * n_j + j]
                )
                projs.append(pj)
        for i in range(n_tok):
            if order == "j_outer":
                for j in range(n_j):
                    ps = psum_pool.tile(
                        [P, 512], f32, tag="ps", name=f"ps_{rep}_{q}_{i}_{j}"
                    )
                    for c in range(n_chunks):
                        nc.tensor.matmul(
                            ps[:],
                            hts[i][:, c, :],
                            projs[j][:, c, :],
                            start=(c == 0),
                            stop=(c == n_chunks - 1),
                        )
                    osb = osb_pool.tile(
                        [P, 512], out_mydt, tag="osb", name=f"osb_{rep}_{q}_{i}_{j}"
                    )
                    drain_copy(osb[:], ps[:])
                    col0 = q * d_pass + j * 512
                    store(out_d[i * P : (i + 1) * P, col0 : col0 + 512], osb[:])
            else:  # c_outer: each lhsT load feeds n_j consecutive matmuls
                pss = [
                    psum_pool.tile(
                        [P, 512], f32, tag=f"ps{j}", name=f"ps_{rep}_{q}_{i}_{j}"
                    )
                    for j in range(n_j)
                ]
                for c in range(n_chunks):
                    for j in range(n_j):
                        nc.tensor.matmul(
                            pss[j][:],
                            hts[i][:, c, :],
                            projs[j][:, c, :],
                            start=(c == 0),
                            stop=(c == n_chunks - 1),
                        )
                for j in range(n_j):
                    osb = osb_pool.tile(
                        [P, 512], out_mydt, tag="osb", name=f"osb_{rep}_{q}_{i}_{j}"
                    )
                    drain_copy(osb[:], pss[j][:])
                    col0 = q * d_pass + j * 512
                    store(out_d[i * P : (i + 1) * P, col0 : col0 + 512], osb[:])


_NC_CACHE = {}


def _get_nc(key, **kwargs):
    if key not in _NC_CACHE:
        _NC_CACHE[key] = build_kernel(**kwargs)
    return _NC_CACHE[key]


def _bigram_indices(input_ids):
    ids = np.asarray(input_ids).astype(np.int64)
    prev = np.concatenate([np.zeros_like(ids[:, :1]), ids[:, :-1]], axis=1)
    return ((prev * HASH_MULT + ids) % BIGRAM_VOCAB).astype(np.int32)


def _prep_proj(proj_w, d_pass=1536):
    """[D, H] f32 -> [n_blk, 128, H/128, 512] bf16 contiguous blocks."""
    Hh = proj_w.shape[1]
    D = proj_w.shape[0]
    n_chunks = Hh // P
    projT = np.ascontiguousarray(np.asarray(proj_w).T)  # [H, D]
    # [H, D] -> [128, n_chunks, D]: partition p holds rows {c*128 + p}
    a = projT.reshape(n_chunks, P, D).transpose(1, 0, 2)
    # -> [n_blk, 128, n_chunks, 512]
    a = a.reshape(P, n_chunks, D // 512, 512).transpose(2, 0, 1, 3)
    return np.ascontiguousarray(a).astype(ml_dtypes.bfloat16)


# Best-known device config (bench-validated). kernel() always uses this.
# Race-bench (32 interleaved rounds, delta-reps 0 vs 16): base 551.6us >
# drain=dve 549.2 > +store_split 545.2 > +proj_ring=sync 542.8us — each
# offload moves work off the PE-adjacent scalar engine / scalar HWDGE ring.
BEST_CONFIG = dict(
    d_pass=1536,
    order="j_outer",
    prologue="v3",
    out_dt="bf16",
    drain="dve",
    store_split=1,
    proj_ring="sync",
)


def kernel(input_ids, table, proj_w):
    global LAST_RESULT
    b, s = input_ids.shape
    n_tokens = b * s
    T = n_tokens // N_CORES
    assert T % P == 0

    nc = _get_nc(("main", T, tuple(sorted(BEST_CONFIG.items()))), T=T, **BEST_CONFIG)

    flat_idx = _bigram_indices(input_ids).reshape(-1)
    table_bf = np.asarray(table, dtype=ml_dtypes.bfloat16)
    projT_prep = _prep_proj(proj_w)

    in_maps = []
    for ci in range(N_CORES):
        sl = flat_idx[ci * T : (ci + 1) * T]
        idx_np = np.ascontiguousarray(sl.reshape(T // P, P).T).astype(np.int32)
        in_maps.append({"idx": idx_np, "table": table_bf, "projT": projT_prep})

    want_trace = bool(int(os.environ.get("KERNEL_TRACE", "0")))
    if not want_trace:
        # This axon build lacks the NTFF profile hook (antenv.axon_hooks);
        # run_bass_kernel_spmd's trace path would crash on import if the
        # environment sets BASS_TRACE. Force the plain execute path.
        os.environ["BASS_NEVER_TRACE"] = "1"
    LAST_RESULT = run_bass_kernel_spmd(
        nc,
        in_maps,
        core_ids=list(range(N_CORES)),
        trace=want_trace,
    )
    out = np.concatenate(
        [np.asarray(r["out"]).astype(np.float32) for r in LAST_RESULT.results], axis=0
    )
    return out.reshape(b, s, MODEL_DIM)

